# revision 1
# baseline (speedup 1.0000x reference)
"""Trainium2 Bass kernel for nn_KANCubic1D (per-channel cubic-spline KAN layer).

out = id_gain[c]*x + spline_c(clamp(a[c]*x+b[c], +-1.5)) + bias[c]

Strategy: data-parallel batch shard across 8 cores. Per core, 3 tiles of
[128 partitions = 64 channels x 2 rows, 8192 cols]. The per-channel
piecewise-cubic spline (33 intervals in v = 15.5*x_aff + 16.5, clamped to
[0, 33)) is evaluated gather-free as three chained custom-DVE sweeps:

  spline(v) = S0 + sum_m dg[m]*relu(v-m)            (Abel-summed linear part)
            + sum_m c2[m]*q_m^2 + c3[m]*q_m^3,      q_m = clamp(v-m, 0, 1)

with per-(channel,bin) coefficients derived host-side from alpha.
"""
import numpy as np

import concourse.bass as bass
import concourse.bacc as bacc
import concourse.mybir as mybir
from concourse import tile
from concourse.bass_utils import run_bass_kernel_spmd
import concourse.dve_ops as dve_ops
from concourse.dve_spec import Spec, Src0, Src1, C2, One, relu, sq, minn, lower, _has_src1
from concourse.dve_spec import C0 as SC0, C1 as SC1
from concourse.dve_uop import DveOpSpec

B, C, H, W, K = 32, 192, 64, 64, 32
NCORES = 8
BLOC = B // NCORES            # 4 batches per core
NBINS = 33
NTILES = 3                    # 64 channels per tile, 2 rows per channel
COLS = BLOC * H * W // 2      # 8192
VMAX = float(np.nextafter(np.float32(33.0), np.float32(0.0)))
NS = 4 + 3 * NBINS            # scalar columns per tile

F32 = mybir.dt.float32
ALU = mybir.AluOpType


# --------------------------------------------------------------- custom ops
def _register(name, spec):
    for op in dve_ops.OPS:
        if op.name == name:
            return op
    row = dve_ops._CUSTOM_DVE_ROW_BASE + len(dve_ops.OPS)
    assert row < 0x20
    shas = {}
    for ver in ("v3", "v4"):
        s = DveOpSpec(name=name, opcode=row, uops=lower(spec, ver=ver),
                      rd1_en=_has_src1(spec))
        shas[ver] = s.sha(ver)
    op = dve_ops.DveOp(name, spec, subdim=False, uops_sha=shas)
    dve_ops.OPS.append(op)
    dve_ops._SUB_OPCODE_FOR_NAME[name] = row
    dve_ops.CUSTOM_DVE_SPECS[name] = spec
    return op


def _q(s):
    return minn(relu(s), One)


def _clip01(y, m):
    return np.minimum(np.maximum(y - m, 0.0), 1.0)


KAN_LIN = _register("KAN_LIN", Spec(
    body=Src0 + SC0 * relu(Src1 - C2),
    reference=lambda in0, in1, s0, s1, imm2:
        in0 + s0 * np.maximum(in1 - imm2, 0.0),
))
KAN_QUAD = _register("KAN_QUAD", Spec(
    body=Src0 + SC0 * sq(_q(Src1 - C2)),
    reference=lambda in0, in1, s0, s1, imm2:
        in0 + s0 * _clip01(in1, imm2) ** 2,
))
KAN_CUBE = _register("KAN_CUBE", Spec(
    body=Src0 + SC0 * (sq(_q(Src1 - C2)) * _q(Src1 - C2)),
    reference=lambda in0, in1, s0, s1, imm2:
        in0 + s0 * _clip01(in1, imm2) ** 3,
))


# ------------------------------------------------------- coefficient tables
def _derive_tables(alpha):
    """alpha [C,K] -> S0 [C], dg [C,33], c2 [C,33], c3 [C,33] (float64)."""
    al = alpha.astype(np.float64)
    m = np.arange(NBINS)
    A = np.stack([al[:, np.clip(m - 2 + j, 0, K - 1)] for j in range(4)])
    q0 = (A[0] + 4 * A[1] + A[2]) / 6.0
    q1 = (A[2] - A[0]) / 2.0
    q2 = (A[0] - 2 * A[1] + A[2]) / 2.0
    q3 = (-A[0] + 3 * A[1] - 3 * A[2] + A[3]) / 6.0
    S0 = q0[:, 0]
    dg = np.concatenate([q1[:, :1], np.diff(q1, axis=1)], axis=1)
    return S0, dg, q2, q3


def _build_scal(a, b, alpha, id_gain, bias):
    S0, dg, c2, c3 = _derive_tables(alpha)
    scal = np.zeros((NTILES, 128, NS), np.float64)
    ch = np.arange(128) // 2  # channel-local per partition
    for t in range(NTILES):
        c = 64 * t + ch
        scal[t, :, 0] = a[c]
        scal[t, :, 1] = b[c]
        scal[t, :, 2] = id_gain[c]
        scal[t, :, 3] = bias[c] + S0[c]
        scal[t, :, 4:4 + NBINS] = dg[c]
        scal[t, :, 4 + NBINS:4 + 2 * NBINS] = c2[c]
        scal[t, :, 4 + 2 * NBINS:4 + 3 * NBINS] = c3[c]
    return np.ascontiguousarray(scal.astype(np.float32))


# ------------------------------------------------------------- bass program
_CACHE = {}


def _build_nc():
    if "nc" in _CACHE:
        return _CACHE["nc"]
    nc = bacc.Bacc("TRN2", target_bir_lowering=False)
    x_d = nc.dram_tensor("x", (BLOC, C, H, W), F32, kind="ExternalInput")
    s_d = nc.dram_tensor("scal", (NTILES, 128, NS), F32, kind="ExternalInput")
    o_d = nc.dram_tensor("out", (BLOC, C, H, W), F32, kind="ExternalOutput")

    with tile.TileContext(nc) as tc:
        with (
            tc.tile_pool(name="xs", bufs=2) as xp,
            tc.tile_pool(name="vs", bufs=2) as vp,
            tc.tile_pool(name="ac", bufs=2) as ap_,
            tc.tile_pool(name="sc", bufs=1) as sp,
        ):
            scal = sp.tile([128, NTILES * NS], F32)
            nc.sync.dma_start(scal[:], s_d.rearrange("t p s -> p t s"))

            for t in range(NTILES):
                def sc(col, _t=t):
                    off = _t * NS + col
                    return scal[:, off:off + 1]

                src = x_d[:, 64 * t:64 * (t + 1), :, :].rearrange(
                    "(r j) c h w -> c r j (h w)", r=2, j=2)
                xt = xp.tile([128, COLS], F32, tag="xt")
                nc.sync.dma_start(xt[:], src)

                vt = vp.tile([128, COLS], F32, tag="vt")
                acc = ap_.tile([128, COLS], F32, tag="acc")
                # w = a*x + b  (into vt);  v = clamp(15.5*w + 16.5, 0, VMAX)
                nc.vector.tensor_scalar(vt[:], xt[:], sc(0), sc(1), ALU.mult, ALU.add)
                nc.vector.tensor_scalar(vt[:], vt[:], 15.5, 16.5, ALU.mult, ALU.add)
                nc.vector.tensor_scalar(vt[:], vt[:], 0.0, VMAX, ALU.max, ALU.min)
                # acc = id_gain*x + (bias + S0)
                nc.vector.tensor_scalar(acc[:], xt[:], sc(2), sc(3), ALU.mult, ALU.add)

                for m in range(NBINS):
                    nc.vector._custom_dve(KAN_QUAD, out=acc[:], in0=acc[:], in1=vt[:],
                                          s0=sc(4 + NBINS + m), s1=0.0, imm2=float(m))
                    nc.vector._custom_dve(KAN_CUBE, out=acc[:], in0=acc[:], in1=vt[:],
                                          s0=sc(4 + 2 * NBINS + m), s1=0.0, imm2=float(m))
                    nc.vector._custom_dve(KAN_LIN, out=acc[:], in0=acc[:], in1=vt[:],
                                          s0=sc(4 + m), s1=0.0, imm2=float(m))

                dst = o_d[:, 64 * t:64 * (t + 1), :, :].rearrange(
                    "(r j) c h w -> c r j (h w)", r=2, j=2)
                nc.sync.dma_start(dst, acc[:])

    nc.compile()
    _CACHE["nc"] = nc
    return nc


# ------------------------------------------------------------------- entry
def kernel(**inputs):
    x = np.ascontiguousarray(np.asarray(inputs["x"], dtype=np.float32))
    a = np.asarray(inputs["a"], np.float64)
    b = np.asarray(inputs["b"], np.float64)
    alpha = np.asarray(inputs["alpha"], np.float64)
    id_gain = np.asarray(inputs["id_gain"], np.float64)
    bias = np.asarray(inputs["bias"], np.float64)

    scal = _build_scal(a, b, alpha, id_gain, bias)
    nc = _build_nc()
    in_maps = [
        {"x": np.ascontiguousarray(x[k * BLOC:(k + 1) * BLOC]), "scal": scal}
        for k in range(NCORES)
    ]
    res = run_bass_kernel_spmd(nc, in_maps, core_ids=list(range(NCORES)))
    outs = []
    for r in res.results:
        out = r["out"] if isinstance(r, dict) else r[0]
        outs.append(np.asarray(out, np.float32).reshape(BLOC, C, H, W))
    return np.concatenate(outs, axis=0)


if __name__ == "__main__":
    rng = np.random.default_rng(0)
    ins = {
        "x": rng.standard_normal((B, C, H, W), dtype=np.float32),
        "a": rng.standard_normal(C).astype(np.float32),
        "b": rng.standard_normal(C).astype(np.float32),
        "alpha": rng.standard_normal((C, K)).astype(np.float32),
        "id_gain": rng.standard_normal(C).astype(np.float32),
        "bias": rng.standard_normal(C).astype(np.float32),
    }
    out = kernel(**ins)
    print("out", out.shape, out.dtype, float(np.abs(out).max()))



# revision 3
# speedup vs baseline: 1.1684x; 1.1684x over previous
"""Trainium2 Bass kernel for nn_KANCubic1D (per-channel cubic-spline KAN layer).

out = id_gain[c]*x + spline_c(clamp(a[c]*x+b[c], +-1.5)) + bias[c]

The spline (uniform cubic B-spline with clamped coefficient indexing) is
globally C2, so on the clamped domain it equals ONE cubic polynomial plus 32
truncated-cubic knot terms. With the centered stream s = clamp(15.5*(a*x+b),
+-16.5) (knots at half-integers mu = m-16.5):

  S = P(s) + sum_{m=17}^{32} c_m relu(s-mu_m)^3 + sum_{m=1}^{16} c_m relu(mu_m-s)^3

where P is piece-16's cubic extended globally and c_m = q3[m]-q3[m-1] (third-
derivative jumps /6). Each knot term is ONE 6-stage custom DVE op (vs 3 ops
per bin in the previous scheme), evaluated two-sided from the middle so fp32
partial sums stay ~1e3.

Data-parallel batch shard across 8 cores; per core 3 tiles of
[128 partitions = 64 channels x 2 rows, 8192 cols].
"""
import numpy as np

import concourse.bass as bass
import concourse.bacc as bacc
import concourse.mybir as mybir
from concourse import tile
from concourse.bass_utils import run_bass_kernel_spmd
import concourse.dve_ops as dve_ops
from concourse.dve_spec import Spec, Src0, Src1, Zero, One, relu, sq, minn, maxx, lower, _has_src1
from concourse.dve_spec import C0 as SC0, C1 as SC1, C2
from concourse.dve_uop import DveOpSpec

B, C, H, W, K = 32, 192, 64, 64, 32
NCORES = 8
BLOC = B // NCORES            # 4 batches per core
NTILES = 3                    # 64 channels per tile, 2 rows per channel
COLS = BLOC * H * W // 2      # 8192
SMAX = 16.5
NS = 7 + 32                   # scalar columns per tile: Aeff Beff p3 p2 p1 p0D G + c[1..32]

F32 = mybir.dt.float32
ALU = mybir.AluOpType


# --------------------------------------------------------------- custom ops
def _register(name, spec):
    for op in dve_ops.OPS:
        if op.name == name:
            return op
    row = dve_ops._CUSTOM_DVE_ROW_BASE + len(dve_ops.OPS)
    assert row < 0x20
    shas = {}
    for ver in ("v3", "v4"):
        s = DveOpSpec(name=name, opcode=row, uops=lower(spec, ver=ver),
                      rd1_en=_has_src1(spec))
        shas[ver] = s.sha(ver)
    op = dve_ops.DveOp(name, spec, subdim=False, uops_sha=shas)
    dve_ops.OPS.append(op)
    dve_ops._SUB_OPCODE_FOR_NAME[name] = row
    dve_ops.CUSTOM_DVE_SPECS[name] = spec
    return op


def _cube(r):
    return sq(r) * r


# s = clamp(s0*x + s1, -imm2, +imm2)
KAN_AFF = _register("KAN_AFF", Spec(
    body=minn(maxx(SC0 * Src0 + SC1, Zero - C2), C2),
    reference=lambda in0, in1, s0, s1, imm2:
        np.minimum(np.maximum(s0 * in0 + s1, -imm2), imm2),
))
# out = in0*in1 + s0   (Horner step)
KAN_FMA = _register("KAN_FMA", Spec(
    body=Src0 * Src1 + SC0,
    reference=lambda in0, in1, s0, s1, imm2: in0 * in1 + s0,
))
# out = in0 + s0*in1   (id_gain*x merge)
KAN_AXPY = _register("KAN_AXPY", Spec(
    body=Src0 + SC0 * Src1,
    reference=lambda in0, in1, s0, s1, imm2: in0 + s0 * in1,
))
# out = in0 + s0*relu(in1 - imm2)^3
KAN_CUBE_R = _register("KAN_CUBE_R", Spec(
    body=Src0 + SC0 * _cube(relu(Src1 - C2)),
    reference=lambda in0, in1, s0, s1, imm2:
        in0 + s0 * np.maximum(in1 - imm2, 0.0) ** 3,
))
# out = in0 + s0*relu(imm2 - in1)^3
KAN_CUBE_L = _register("KAN_CUBE_L", Spec(
    body=Src0 + SC0 * _cube(relu(C2 - Src1)),
    reference=lambda in0, in1, s0, s1, imm2:
        in0 + s0 * np.maximum(imm2 - in1, 0.0) ** 3,
))


# ------------------------------------------------------- coefficient tables
def _derive_tables(alpha):
    """alpha [C,K] -> P coeffs p0..p3 [C] (in s = v-16.5) and knot jumps c [C,33]."""
    al = alpha.astype(np.float64)
    m = np.arange(33)
    A = np.stack([al[:, np.clip(m - 2 + j, 0, K - 1)] for j in range(4)])
    q0 = (A[0] + 4 * A[1] + A[2]) / 6.0
    q1 = (A[2] - A[0]) / 2.0
    q2 = (A[0] - 2 * A[1] + A[2]) / 2.0
    q3 = (-A[0] + 3 * A[1] - 3 * A[2] + A[3]) / 6.0
    c = np.concatenate([q3[:, :1], np.diff(q3, axis=1)], axis=1)
    M0, t0 = 16, 0.5  # piece 16 recentered at s = t16 - 0.5
    p3 = q3[:, M0]
    p2 = q2[:, M0] + 3 * p3 * t0
    p1 = q1[:, M0] + 2 * q2[:, M0] * t0 + 3 * p3 * t0 * t0
    p0 = q0[:, M0] + q1[:, M0] * t0 + q2[:, M0] * t0 ** 2 + p3 * t0 ** 3
    return p0, p1, p2, p3, c


def _build_scal(a, b, alpha, id_gain, bias):
    p0, p1, p2, p3, c = _derive_tables(alpha)
    scal = np.zeros((NTILES, 128, NS), np.float64)
    ch = np.arange(128) // 2  # channel-local per partition
    for t in range(NTILES):
        cc = 64 * t + ch
        scal[t, :, 0] = 15.5 * a[cc]
        scal[t, :, 1] = 15.5 * b[cc]
        scal[t, :, 2] = p3[cc]
        scal[t, :, 3] = p2[cc]
        scal[t, :, 4] = p1[cc]
        scal[t, :, 5] = p0[cc] + bias[cc]
        scal[t, :, 6] = id_gain[cc]
        scal[t, :, 7:7 + 32] = c[cc][:, 1:33]
    return np.ascontiguousarray(scal.astype(np.float32))


# ------------------------------------------------------------- bass program
_CACHE = {}


def _build_nc():
    if "nc" in _CACHE:
        return _CACHE["nc"]
    nc = bacc.Bacc("TRN2", target_bir_lowering=False)
    x_d = nc.dram_tensor("x", (BLOC, C, H, W), F32, kind="ExternalInput")
    s_d = nc.dram_tensor("scal", (NTILES, 128, NS), F32, kind="ExternalInput")
    o_d = nc.dram_tensor("out", (BLOC, C, H, W), F32, kind="ExternalOutput")

    with tile.TileContext(nc) as tc:
        with (
            tc.tile_pool(name="xs", bufs=2) as xp,
            tc.tile_pool(name="vs", bufs=1) as vp,
            tc.tile_pool(name="ac", bufs=2) as ap_,
            tc.tile_pool(name="sc", bufs=1) as sp,
        ):
            scal = sp.tile([128, NTILES * NS], F32)
            nc.sync.dma_start(scal[:], s_d.rearrange("t p s -> p t s"))

            for t in range(NTILES):
                def sc(col, _t=t):
                    off = _t * NS + col
                    return scal[:, off:off + 1]

                src = x_d[:, 64 * t:64 * (t + 1), :, :].rearrange(
                    "(r j) c h w -> c r j (h w)", r=2, j=2)
                xt = xp.tile([128, COLS], F32, tag="xt")
                nc.sync.dma_start(xt[:], src)

                st = vp.tile([128, COLS], F32, tag="st")
                acc = ap_.tile([128, COLS], F32, tag="acc")
                # s = clamp(Aeff*x + Beff, +-SMAX)
                nc.vector._custom_dve(KAN_AFF, out=st[:], in0=xt[:],
                                      s0=sc(0), s1=sc(1), imm2=SMAX)
                # acc = P(s) via Horner
                nc.vector.tensor_scalar(acc[:], st[:], sc(2), sc(3), ALU.mult, ALU.add)
                nc.vector._custom_dve(KAN_FMA, out=acc[:], in0=acc[:], in1=st[:],
                                      s0=sc(4), s1=0.0, imm2=0.0)
                nc.vector._custom_dve(KAN_FMA, out=acc[:], in0=acc[:], in1=st[:],
                                      s0=sc(5), s1=0.0, imm2=0.0)
                # acc += id_gain*x
                nc.vector._custom_dve(KAN_AXPY, out=acc[:], in0=acc[:], in1=xt[:],
                                      s0=sc(6), s1=0.0, imm2=0.0)
                # 32 knot terms, two-sided
                for m in range(17, 33):
                    nc.vector._custom_dve(KAN_CUBE_R, out=acc[:], in0=acc[:], in1=st[:],
                                          s0=sc(7 + m - 1), s1=0.0, imm2=float(m) - 16.5)
                for m in range(1, 17):
                    nc.vector._custom_dve(KAN_CUBE_L, out=acc[:], in0=acc[:], in1=st[:],
                                          s0=sc(7 + m - 1), s1=0.0, imm2=float(m) - 16.5)

                dst = o_d[:, 64 * t:64 * (t + 1), :, :].rearrange(
                    "(r j) c h w -> c r j (h w)", r=2, j=2)
                nc.sync.dma_start(dst, acc[:])

    nc.compile()
    _CACHE["nc"] = nc
    return nc


# ------------------------------------------------------------------- entry
def kernel(**inputs):
    x = np.ascontiguousarray(np.asarray(inputs["x"], dtype=np.float32))
    a = np.asarray(inputs["a"], np.float64)
    b = np.asarray(inputs["b"], np.float64)
    alpha = np.asarray(inputs["alpha"], np.float64)
    id_gain = np.asarray(inputs["id_gain"], np.float64)
    bias = np.asarray(inputs["bias"], np.float64)

    scal = _build_scal(a, b, alpha, id_gain, bias)
    nc = _build_nc()
    in_maps = [
        {"x": np.ascontiguousarray(x[k * BLOC:(k + 1) * BLOC]), "scal": scal}
        for k in range(NCORES)
    ]
    res = run_bass_kernel_spmd(nc, in_maps, core_ids=list(range(NCORES)))
    outs = []
    for r in res.results:
        out = r["out"] if isinstance(r, dict) else r[0]
        outs.append(np.asarray(out, np.float32).reshape(BLOC, C, H, W))
    return np.concatenate(outs, axis=0)


if __name__ == "__main__":
    rng = np.random.default_rng(0)
    ins = {
        "x": rng.standard_normal((B, C, H, W), dtype=np.float32),
        "a": rng.standard_normal(C).astype(np.float32),
        "b": rng.standard_normal(C).astype(np.float32),
        "alpha": rng.standard_normal((C, K)).astype(np.float32),
        "id_gain": rng.standard_normal(C).astype(np.float32),
        "bias": rng.standard_normal(C).astype(np.float32),
    }
    out = kernel(**ins)
    print("out", out.shape, out.dtype, float(np.abs(out).max()))


# revision 4
# speedup vs baseline: 1.2046x; 1.0310x over previous
"""Trainium2 Bass kernel for nn_KANCubic1D — instruction-count + DMA-overlap optimized.

Math identical to kernel_v2 (two-sided truncated-power cubic spline):
  s = clamp(15.5*(a*x+b), +-16.5)
  out = id_gain*x + (p0+bias) + p1*s + s^2*(p2 + p3*s)
        + sum_{mu in +-{0.5..15.5}} c_mu * relu(+-(s - mu))^3

This environment pays a large FIXED cost per engine instruction (~42us on
DVE, measured), so instruction count dominates.  vs kernel_v2 (3 tiles x 37
ops = 111):
  - 2 tiles instead of 3: [128ch x 1row x 16384] + [64ch x 2rows x 8192]
  - merged ops: INIT (id_gain*x + p1*s, 3 stages), P23 (s^2*(p2+p3*s),
    5 stages), p0+bias rides a knot op's spare scalar (8 stages)
  -> 35 ops/tile, 70 total.
"""
import numpy as np

import concourse.bass as bass
import concourse.bacc as bacc
import concourse.mybir as mybir
from concourse import tile
from concourse.bass_utils import run_bass_kernel_spmd
import concourse.dve_ops as dve_ops
from concourse.dve_spec import Spec, Src0, Src1, Zero, One, relu, sq, minn, maxx, lower, _has_src1
from concourse.dve_spec import C0 as SC0, C1 as SC1, C2
from concourse.dve_uop import DveOpSpec

B, C, H, W, K = 32, 192, 64, 64, 32
NCORES = 8
BLOC = B // NCORES            # 4
COLS_A = BLOC * H * W         # 16384 (tile A: 128 channels, 1 row each)
COLS_B = BLOC * H * W // 2    # 8192  (tile B: 64 channels, 2 rows each)
SMAX = 16.5
NS = 7 + 32

F32 = mybir.dt.float32
ALU = mybir.AluOpType


def _register(name, spec):
    for op in dve_ops.OPS:
        if op.name == name:
            return op
    row = dve_ops._CUSTOM_DVE_ROW_BASE + len(dve_ops.OPS)
    assert row < 0x20
    shas = {}
    for ver in ("v3", "v4"):
        s = DveOpSpec(name=name, opcode=row, uops=lower(spec, ver=ver),
                      rd1_en=_has_src1(spec))
        shas[ver] = s.sha(ver)
    op = dve_ops.DveOp(name, spec, subdim=False, uops_sha=shas)
    dve_ops.OPS.append(op)
    dve_ops._SUB_OPCODE_FOR_NAME[name] = row
    dve_ops.CUSTOM_DVE_SPECS[name] = spec
    return op


def _cube(r):
    return sq(r) * r


# s = clamp(s0*x + s1, -imm2, +imm2)
KAN_AFF = _register("KAN_AFF", Spec(
    body=minn(maxx(SC0 * Src0 + SC1, Zero - C2), C2),
    reference=lambda in0, in1, s0, s1, imm2:
        np.minimum(np.maximum(s0 * in0 + s1, -imm2), imm2),
))
# acc = s0*x + s1*s
KAN_INIT = _register("KAN_INIT", Spec(
    body=SC0 * Src0 + SC1 * Src1,
    reference=lambda in0, in1, s0, s1, imm2: s0 * in0 + s1 * in1,
))
# acc += s^2*(s0 + s1*s)
KAN_P23 = _register("KAN_P23", Spec(
    body=Src0 + sq(Src1) * (SC0 + SC1 * Src1),
    reference=lambda in0, in1, s0, s1, imm2: in0 + in1 * in1 * (s0 + s1 * in1),
))
# acc += s0*relu(s - imm2)^3
KAN_CUBE_R = _register("KAN_CUBE_R", Spec(
    body=Src0 + SC0 * _cube(relu(Src1 - C2)),
    reference=lambda in0, in1, s0, s1, imm2:
        in0 + s0 * np.maximum(in1 - imm2, 0.0) ** 3,
))
# acc += s0*relu(imm2 - s)^3
KAN_CUBE_L = _register("KAN_CUBE_L", Spec(
    body=Src0 + SC0 * _cube(relu(C2 - Src1)),
    reference=lambda in0, in1, s0, s1, imm2:
        in0 + s0 * np.maximum(imm2 - in1, 0.0) ** 3,
))
# acc += s0*relu(imm2 - s)^3 + s1   (bias rider)
KAN_CUBE_LB = _register("KAN_CUBE_LB", Spec(
    body=Src0 + SC0 * _cube(relu(C2 - Src1)) + SC1,
    reference=lambda in0, in1, s0, s1, imm2:
        in0 + s0 * np.maximum(imm2 - in1, 0.0) ** 3 + s1,
))


def _derive_tables(alpha):
    """p0..p3 [C] (cubic in centered s = v-16.5) and knot jumps c [C,33]."""
    al = alpha.astype(np.float64)
    m = np.arange(33)
    A = np.stack([al[:, np.clip(m - 2 + j, 0, K - 1)] for j in range(4)])
    q0 = (A[0] + 4 * A[1] + A[2]) / 6.0
    q1 = (A[2] - A[0]) / 2.0
    q2 = (A[0] - 2 * A[1] + A[2]) / 2.0
    q3 = (-A[0] + 3 * A[1] - 3 * A[2] + A[3]) / 6.0
    c = np.concatenate([q3[:, :1], np.diff(q3, axis=1)], axis=1)
    M0, t0 = 16, 0.5
    p3 = q3[:, M0]
    p2 = q2[:, M0] + 3 * p3 * t0
    p1 = q1[:, M0] + 2 * q2[:, M0] * t0 + 3 * p3 * t0 * t0
    p0 = q0[:, M0] + q1[:, M0] * t0 + q2[:, M0] * t0 ** 2 + p3 * t0 ** 3
    return p0, p1, p2, p3, c


def _build_scal(a, b, alpha, id_gain, bias):
    p0, p1, p2, p3, c = _derive_tables(alpha)
    scal = np.zeros((2, 128, NS), np.float64)
    cc_a = np.arange(128)                 # tile A: channel = partition
    cc_b = 128 + np.arange(128) // 2      # tile B: 2 rows per channel
    for t, cc in ((0, cc_a), (1, cc_b)):
        scal[t, :, 0] = 15.5 * a[cc]
        scal[t, :, 1] = 15.5 * b[cc]
        scal[t, :, 2] = id_gain[cc]
        scal[t, :, 3] = p1[cc]
        scal[t, :, 4] = p2[cc]
        scal[t, :, 5] = p3[cc]
        scal[t, :, 6] = p0[cc] + bias[cc]
        scal[t, :, 7:7 + 32] = c[cc][:, 1:33]
    return np.ascontiguousarray(scal.astype(np.float32))


_CACHE = {}


def _emit_tile(nc, sc, xt, st, acc, cols):
    """35 DVE ops for one tile."""
    nc.vector._custom_dve(KAN_AFF, out=st, in0=xt,
                          s0=sc(0), s1=sc(1), imm2=SMAX)
    nc.vector._custom_dve(KAN_INIT, out=acc, in0=xt, in1=st,
                          s0=sc(2), s1=sc(3), imm2=0.0)
    nc.vector._custom_dve(KAN_P23, out=acc, in0=acc, in1=st,
                          s0=sc(4), s1=sc(5), imm2=0.0)
    # left knot mu=-0.5 carries p0+bias on its spare scalar
    nc.vector._custom_dve(KAN_CUBE_LB, out=acc, in0=acc, in1=st,
                          s0=sc(7 + 16 - 1), s1=sc(6), imm2=-0.5)
    for m in range(1, 16):     # left knots m=1..15 -> mu = m-16.5
        nc.vector._custom_dve(KAN_CUBE_L, out=acc, in0=acc, in1=st,
                              s0=sc(7 + m - 1), s1=0.0, imm2=float(m) - 16.5)
    for m in range(17, 33):    # right knots -> mu = m-16.5
        nc.vector._custom_dve(KAN_CUBE_R, out=acc, in0=acc, in1=st,
                              s0=sc(7 + m - 1), s1=0.0, imm2=float(m) - 16.5)


def _build_nc():
    if "nc" in _CACHE:
        return _CACHE["nc"]
    nc = bacc.Bacc("TRN2", target_bir_lowering=False)
    x_d = nc.dram_tensor("x", (BLOC, C, H, W), F32, kind="ExternalInput")
    s_d = nc.dram_tensor("scal", (2, 128, NS), F32, kind="ExternalInput")
    o_d = nc.dram_tensor("out", (BLOC, C, H, W), F32, kind="ExternalOutput")

    with tile.TileContext(nc) as tc:
        with (
            tc.tile_pool(name="xs", bufs=1) as xp,
            tc.tile_pool(name="vs", bufs=1) as vp,
            tc.tile_pool(name="ac", bufs=1) as ap_,
            tc.tile_pool(name="sc", bufs=1) as sp,
        ):
            scal = sp.tile([128, 2 * NS], F32)
            nc.sync.dma_start(scal[:], s_d.rearrange("t p s -> p t s"))

            # ---- tile A: channels 0..127, partition = channel
            def sc_a(col):
                return scal[:, col:col + 1]
            src_a = x_d[:, 0:128, :, :].rearrange("b c h w -> c b (h w)")
            xa = xp.tile([128, COLS_A], F32, tag="xa")
            nc.sync.dma_start(xa[:], src_a)
            sa = vp.tile([128, COLS_A], F32, tag="sa")
            aa = ap_.tile([128, COLS_A], F32, tag="aa")
            _emit_tile(nc, sc_a, xa[:], sa[:], aa[:], COLS_A)
            dst_a = o_d[:, 0:128, :, :].rearrange("b c h w -> c b (h w)")
            nc.sync.dma_start(dst_a, aa[:])

            # ---- tile B: channels 128..191, 2 rows per channel.
            # Buffers alias tile A's dead space so the B input DMA overlaps
            # tile A compute and the A output DMA overlaps tile B compute:
            #   xb = tail of xa (xa fully consumed by INIT at op 2)
            #   sb = tail of sa, ab = head of sa (sa's last reader is tile A's
            #   final knot op; DVE in-order makes the WAR free)
            def sc_b(col):
                return scal[:, NS + col:NS + col + 1]
            src_b = x_d[:, 128:192, :, :].rearrange(
                "(r j) c h w -> c r j (h w)", r=2, j=2)
            xb = xa[:, COLS_B:COLS_A]
            sb = sa[:, COLS_B:COLS_A]
            ab = sa[:, 0:COLS_B]
            nc.sync.dma_start(xb, src_b)
            _emit_tile(nc, sc_b, xb, sb, ab, COLS_B)
            dst_b = o_d[:, 128:192, :, :].rearrange(
                "(r j) c h w -> c r j (h w)", r=2, j=2)
            nc.sync.dma_start(dst_b, ab[:])

    nc.compile()
    _CACHE["nc"] = nc
    return nc


def kernel(**inputs):
    x = np.ascontiguousarray(np.asarray(inputs["x"], dtype=np.float32))
    a = np.asarray(inputs["a"], np.float64)
    b = np.asarray(inputs["b"], np.float64)
    alpha = np.asarray(inputs["alpha"], np.float64)
    id_gain = np.asarray(inputs["id_gain"], np.float64)
    bias = np.asarray(inputs["bias"], np.float64)

    scal = _build_scal(a, b, alpha, id_gain, bias)
    nc = _build_nc()
    in_maps = [
        {"x": np.ascontiguousarray(x[k * BLOC:(k + 1) * BLOC]), "scal": scal}
        for k in range(NCORES)
    ]
    res = run_bass_kernel_spmd(nc, in_maps, core_ids=list(range(NCORES)))
    outs = []
    for r in res.results:
        out = r["out"] if isinstance(r, dict) else r[0]
        outs.append(np.asarray(out, np.float32).reshape(BLOC, C, H, W))
    return np.concatenate(outs, axis=0)


if __name__ == "__main__":
    rng = np.random.default_rng(0)
    ins = {
        "x": rng.standard_normal((B, C, H, W), dtype=np.float32),
        "a": rng.standard_normal(C).astype(np.float32),
        "b": rng.standard_normal(C).astype(np.float32),
        "alpha": rng.standard_normal((C, K)).astype(np.float32),
        "id_gain": rng.standard_normal(C).astype(np.float32),
        "bias": rng.standard_normal(C).astype(np.float32),
    }
    out = kernel(**ins)
    print("out", out.shape, out.dtype, float(np.abs(out).max()))
